# revision 9
# baseline (speedup 1.0000x reference)
"""Trainium2 Bass kernel for nn_BasicBlockBi (TBN basic block, 2x ternary-binary conv).

Strategy: data-parallel over batch (4 images per core on 8 cores).
  - BN + ternarize thresholds for block 1 are folded on host into per-channel
    compare thresholds (delta1 computed on host from the full input).
  - Ternary activations {-1,0,+1} and binary sign weights are exact in bf16,
    so convs run as 3x3-tap shifted matmuls accumulating exact integers in PSUM;
    the per-output-channel alpha scale + shortcut add are fused in one DVE op.
  - delta2 = 0.7*mean|bn2(h)| needs a global mean over the full batch: per-core
    partial sums are AllReduced across the 8 cores in-kernel.
"""

import os
import sys

for _p in ("/opt/trn_rl_repo", "/root/.axon_site/_ro/trn_rl_repo"):
    if os.path.isdir(_p) and _p not in sys.path:
        sys.path.append(_p)

import numpy as np

import concourse.bacc as bacc
import concourse.bass as bass
import concourse.tile as tile
from concourse import bass_isa, mybir
from concourse.bass_utils import run_bass_kernel_spmd

B, C, H, W = 32, 256, 32, 32
HW = H * W
NCORES = 8
BL = B // NCORES          # images per core
CCH = C // 128            # channel chunks of 128
PW = W + 2                # padded row width
PBUF = 1 + PW * PW        # lead zero + 34x34 padded plane
PBUF_AL = PBUF + 3        # tail pad so over-slices stay in bounds
EPS = 1e-5
FRAC = 0.7

QDT = mybir.dt.bfloat16   # matmul dtype for ternary/sign values (exact)

AOP = mybir.AluOpType
AFT = mybir.ActivationFunctionType
F32 = mybir.dt.float32

# vecs rows
V_T1HI, V_T1LO, V_A1, V_A2, V_S2, V_B2, V_RS2, V_NRS2 = range(8)
NVEC = 8

TRACE = False
LAST_RESULT = None

_cache: dict = {}


def _ternarize(nc, vec_hi, vec_lo, src, qtile, ci, tmp_pool):
    """Write ternary {-1,0,+1} of src (128,HW) into padded plane qtile[...] interior.

    qtile interior layout: position 1 + PW*(y+1) + x holds data element (y, x);
    borders (lead zero, rows 0/33, cols 32/33) must already be zero.
    """
    t1 = tmp_pool.tile([128, HW], QDT, tag="t1")
    t2 = tmp_pool.tile([128, HW], QDT, tag="t2")
    nc.vector.tensor_scalar(t1, src, vec_hi, None, AOP.is_gt)
    nc.gpsimd.tensor_scalar(t2, src, vec_lo, None, AOP.is_lt)
    qint = qtile[:, 1 + PW : 1 + PW + 32 * PW].rearrange(
        "p (r c) -> p r c", c=PW
    )[:, :, 0:32]
    t1r = t1.rearrange("p (r c) -> p r c", c=32)
    t2r = t2.rearrange("p (r c) -> p r c", c=32)
    nc.vector.tensor_tensor(qint, t1r, t2r, AOP.subtract)


def _zero_borders(nc, qtile):
    nc.gpsimd.memset(qtile[:, 0 : PW + 1], 0.0)                       # lead zero + row 0
    nc.gpsimd.memset(qtile[:, 1 + PW * 33 : 1 + PW * 34], 0.0)        # row 33
    side = qtile[:, 1 + PW : 1 + PW + 32 * PW].rearrange(
        "p (r c) -> p r c", c=PW
    )[:, :, 32:34]
    nc.gpsimd.memset(side, 0.0)                                       # cols 32/33


def _conv_matmuls(nc, psum_tile, wtiles, qp, co, half):
    """Accumulate the 18 shifted-tap matmuls for one (co chunk, row half)."""
    idx = 0
    for kh in range(3):
        for kw in range(3):
            off = PW * (16 * half + kh) + kw
            for ci in range(CCH):
                rhs = qp[ci][:, off : off + PW * 16].rearrange(
                    "p (r c) -> p r c", c=PW
                )[:, :, 0:32]
                nc.tensor.matmul(
                    psum_tile,
                    lhsT=wtiles[kh * 3 + kw, ci][:, co * 128 : (co + 1) * 128],
                    rhs=rhs,
                    start=(idx == 0),
                    stop=(idx == 17),
                )
                idx += 1


def _build():
    if "nc" in _cache:
        return _cache["nc"]

    nc = bacc.Bacc("TRN2", num_devices=NCORES)

    x_in = nc.dram_tensor("x", (BL, CCH, 128, HW), F32, kind="ExternalInput")
    w1t = nc.dram_tensor("w1t", (9, CCH, 128, C), QDT, kind="ExternalInput")
    w2t = nc.dram_tensor("w2t", (9, CCH, 128, C), QDT, kind="ExternalInput")
    vecs = nc.dram_tensor("vecs", (NVEC, CCH, 128, 1), F32, kind="ExternalInput")
    out_d = nc.dram_tensor("out", (BL, CCH, 128, HW), F32, kind="ExternalOutput")
    cc_in = nc.dram_tensor("cc_in", (128, 1), F32)
    cc_out = nc.dram_tensor("cc_out", (128, 1), F32, addr_space="Shared")
    dsc = nc.dram_tensor("dsc", (1, 1), F32)

    with tile.TileContext(nc) as tc:
        with (
            tc.tile_pool(name="consts", bufs=1) as consts,
            tc.tile_pool(name="persist", bufs=1) as persist,
            tc.tile_pool(name="qpool", bufs=2) as qpool,
            tc.tile_pool(name="tmp", bufs=3) as tmp,
            tc.tile_pool(name="epi", bufs=4) as epi,
            tc.tile_pool(name="psum", bufs=8, space="PSUM") as psum,
        ):
            # ---- constants ----
            vt = {}
            for i in range(NVEC):
                for ci in range(CCH):
                    v = consts.tile([128, 1], F32, tag=f"v{i}_{ci}")
                    nc.sync.dma_start(out=v, in_=vecs[i, ci])
                    vt[i, ci] = v

            w1s, w2s = {}, {}
            for tap in range(9):
                for ci in range(CCH):
                    a = consts.tile([128, C], QDT, tag=f"w1_{tap}_{ci}")
                    nc.sync.dma_start(out=a, in_=w1t[tap, ci])
                    w1s[tap, ci] = a
                    b = consts.tile([128, C], QDT, tag=f"w2_{tap}_{ci}")
                    nc.sync.dma_start(out=b, in_=w2t[tap, ci])
                    w2s[tap, ci] = b

            partials = consts.tile([128, BL * CCH * 2], F32, tag="partials")

            xt, ht = {}, {}

            # ---------- phase A: block 1 + |bn2(h)| partial sums ----------
            pcol = 0
            for n in range(BL):
                for ci in range(CCH):
                    xtile = persist.tile([128, HW], F32, tag=f"x{n}_{ci}")
                    nc.sync.dma_start(out=xtile, in_=x_in[n, ci])
                    xt[n, ci] = xtile

                qp = {}
                for ci in range(CCH):
                    q = qpool.tile([128, PBUF_AL], QDT, tag=f"qp{ci}")
                    _zero_borders(nc, q)
                    _ternarize(nc, vt[V_T1HI, ci], vt[V_T1LO, ci], xt[n, ci], q, ci, tmp)
                    qp[ci] = q

                for co in range(CCH):
                    htile = persist.tile([128, HW], F32, tag=f"h{n}_{co}")
                    ht[n, co] = htile
                    for half in range(2):
                        ps = psum.tile([128, 512], F32, tag="ps")
                        _conv_matmuls(nc, ps, w1s, qp, co, half)
                        sl = slice(half * 512, (half + 1) * 512)
                        # h = alpha1 * conv + x   (one DVE op)
                        nc.vector.scalar_tensor_tensor(
                            out=htile[:, sl],
                            in0=ps,
                            scalar=vt[V_A1, co],
                            in1=xt[n, co][:, sl],
                            op0=AOP.mult,
                            op1=AOP.add,
                        )
                        # |bn2(h)| with per-partition running sum for delta2
                        zabs = tmp.tile([128, 512], F32, tag="zabs")
                        nc.scalar.activation(
                            out=zabs,
                            in_=htile[:, sl],
                            func=AFT.Abs,
                            bias=vt[V_B2, co],
                            scale=vt[V_S2, co],
                            accum_out=partials[:, pcol : pcol + 1],
                        )
                        pcol += 1

            # ---------- delta2 via cross-core AllReduce ----------
            ptot = consts.tile([128, 1], F32, tag="ptot")
            nc.vector.tensor_reduce(ptot, partials, axis=mybir.AxisListType.X, op=AOP.add)
            nc.sync.dma_start(out=cc_in[:], in_=ptot)
            nc.gpsimd.collective_compute(
                "AllReduce",
                AOP.add,
                replica_groups=[list(range(NCORES))],
                ins=[cc_in[:]],
                outs=[cc_out[:]],
            )
            # bring the 128 AllReduced values into one partition row, reduce to a
            # scalar, scale into delta2, then DMA-broadcast to all partitions
            red = consts.tile([1, 128], F32, tag="red")
            nc.sync.dma_start(out=red, in_=cc_out[:])
            drow = consts.tile([1, 1], F32, tag="drow")
            nc.vector.tensor_reduce(drow, red, axis=mybir.AxisListType.X, op=AOP.add)
            nc.vector.tensor_scalar(drow, drow, float(FRAC / (B * C * HW)), None, AOP.mult)
            nc.sync.dma_start(out=dsc[:], in_=drow)
            d2 = consts.tile([128, 1], F32, tag="d2")
            nc.sync.dma_start(out=d2, in_=dsc[:].to_broadcast((128, 1)))

            # per-chunk thresholds: t2hi = (d2 - b2)/s2 ; t2lo = (-d2 - b2)/s2
            t2hi, t2lo = {}, {}
            for ci in range(CCH):
                thi = consts.tile([128, 1], F32, tag=f"t2hi{ci}")
                nc.vector.tensor_tensor(thi, d2, vt[V_B2, ci], AOP.subtract)
                nc.vector.tensor_tensor(thi, thi, vt[V_RS2, ci], AOP.mult)
                t2hi[ci] = thi
                tlo = consts.tile([128, 1], F32, tag=f"t2lo{ci}")
                nc.vector.tensor_tensor(tlo, d2, vt[V_B2, ci], AOP.add)
                nc.vector.tensor_tensor(tlo, tlo, vt[V_NRS2, ci], AOP.mult)
                t2lo[ci] = tlo

            # ---------- phase B: block 2 ----------
            for n in range(BL):
                qp = {}
                for ci in range(CCH):
                    q = qpool.tile([128, PBUF_AL], QDT, tag=f"qpb{ci}")
                    _zero_borders(nc, q)
                    _ternarize(nc, t2hi[ci], t2lo[ci], ht[n, ci], q, ci, tmp)
                    qp[ci] = q

                for co in range(CCH):
                    for half in range(2):
                        ps = psum.tile([128, 512], F32, tag="ps")
                        _conv_matmuls(nc, ps, w2s, qp, co, half)
                        sl = slice(half * 512, (half + 1) * 512)
                        ot = epi.tile([128, 512], F32, tag="ot")
                        nc.vector.scalar_tensor_tensor(
                            out=ot,
                            in0=ps,
                            scalar=vt[V_A2, co],
                            in1=ht[n, co][:, sl],
                            op0=AOP.mult,
                            op1=AOP.add,
                        )
                        nc.sync.dma_start(out=out_d[n, co][:, sl], in_=ot)

    nc.finalize()
    _cache["nc"] = nc
    return nc


def _host_prep(x, w1, w2, gamma1, beta1, mean1, var1, gamma2, beta2, mean2, var2):
    f64 = np.float64
    npq = mybir.dt.np(QDT)

    s1 = (gamma1.astype(f64) / np.sqrt(var1.astype(f64) + EPS))
    b1 = beta1.astype(f64) - mean1.astype(f64) * s1
    assert (s1 > 0).all(), "kernel assumes positive bn scale (gamma>0)"
    # delta1 on host (f64 accumulate)
    z1 = x.astype(f64) * s1[None, :, None, None] + b1[None, :, None, None]
    d1 = FRAC * np.abs(z1).mean()
    t1hi = ((d1 - b1) / s1).astype(np.float32)
    t1lo = ((-d1 - b1) / s1).astype(np.float32)

    s2 = (gamma2.astype(f64) / np.sqrt(var2.astype(f64) + EPS))
    b2 = beta2.astype(f64) - mean2.astype(f64) * s2
    assert (s2 > 0).all(), "kernel assumes positive bn scale (gamma>0)"

    a1 = np.abs(w1.astype(f64)).mean(axis=(1, 2, 3)).astype(np.float32)
    a2 = np.abs(w2.astype(f64)).mean(axis=(1, 2, 3)).astype(np.float32)

    def wsign_t(w):
        # (O, I, 3, 3) -> (9, CCH, 128, C) with [tap, ci_chunk, k, co] = sign(w[co, ci, kh, kw])
        s = np.sign(w).astype(npq)
        return np.ascontiguousarray(
            s.transpose(2, 3, 1, 0).reshape(9, CCH, 128, C)
        )

    w1t = wsign_t(w1)
    w2t = wsign_t(w2)

    vecs = np.zeros((NVEC, CCH, 128, 1), np.float32)
    vecs[V_T1HI] = t1hi.reshape(CCH, 128, 1)
    vecs[V_T1LO] = t1lo.reshape(CCH, 128, 1)
    vecs[V_A1] = a1.reshape(CCH, 128, 1)
    vecs[V_A2] = a2.reshape(CCH, 128, 1)
    vecs[V_S2] = s2.astype(np.float32).reshape(CCH, 128, 1)
    vecs[V_B2] = b2.astype(np.float32).reshape(CCH, 128, 1)
    vecs[V_RS2] = (1.0 / s2).astype(np.float32).reshape(CCH, 128, 1)
    vecs[V_NRS2] = (-1.0 / s2).astype(np.float32).reshape(CCH, 128, 1)
    return w1t, w2t, vecs


def make_in_maps(**inputs):
    x = np.ascontiguousarray(inputs["x"], np.float32)
    w1t, w2t, vecs = _host_prep(
        x,
        np.asarray(inputs["w1"], np.float32),
        np.asarray(inputs["w2"], np.float32),
        *[np.asarray(inputs[k], np.float32) for k in (
            "gamma1", "beta1", "mean1", "var1",
            "gamma2", "beta2", "mean2", "var2",
        )],
    )
    in_maps = []
    for i in range(NCORES):
        xs = np.ascontiguousarray(
            x[i * BL : (i + 1) * BL].reshape(BL, CCH, 128, HW)
        )
        in_maps.append({"x": xs, "w1t": w1t, "w2t": w2t, "vecs": vecs})
    return in_maps


def kernel(**inputs) -> np.ndarray:
    global LAST_RESULT
    nc = _build()
    in_maps = make_in_maps(**inputs)
    res = run_bass_kernel_spmd(nc, in_maps, list(range(NCORES)), trace=TRACE)
    LAST_RESULT = res
    out = np.concatenate(
        [res.results[i]["out"].reshape(BL, C, H, W) for i in range(NCORES)], axis=0
    )
    return out.astype(np.float32, copy=False)


# revision 11
# speedup vs baseline: 1.5676x; 1.5676x over previous
"""Trainium2 Bass kernel for nn_BasicBlockBi (TBN basic block, 2x ternary-binary conv).

Strategy: data-parallel over batch (4 images per core on 8 cores).
  - BN + ternarize thresholds for block 1 are folded on host into per-channel
    compare thresholds (delta1 computed on host from the full input).
  - Ternary activations {-1,0,+1} and binary sign weights are exact in bf16,
    so convs run as 3x3-tap shifted matmuls accumulating exact integers in PSUM;
    the per-output-channel alpha scale + shortcut add are fused in one DVE op.
  - delta2 = 0.7*mean|bn2(h)| needs a global mean over the full batch: per-core
    partial sums are AllReduced across the 8 cores in-kernel.
"""

import os
import sys

for _p in ("/opt/trn_rl_repo", "/root/.axon_site/_ro/trn_rl_repo"):
    if os.path.isdir(_p) and _p not in sys.path:
        sys.path.append(_p)

import numpy as np

import concourse.bacc as bacc
import concourse.bass as bass
import concourse.tile as tile
from concourse import bass_isa, mybir
from concourse.bass_utils import run_bass_kernel_spmd

B, C, H, W = 32, 256, 32, 32
HW = H * W
NCORES = 8
BL = B // NCORES          # images per core
CCH = C // 128            # channel chunks of 128
PW = W + 2                # padded row width
PBUF = 1 + PW * PW        # lead zero + 34x34 padded plane
PBUF_AL = PBUF + 3        # tail pad so over-slices stay in bounds
EPS = 1e-5
FRAC = 0.7

QDT = mybir.dt.bfloat16   # matmul dtype for ternary/sign values (exact)

AOP = mybir.AluOpType
AFT = mybir.ActivationFunctionType
F32 = mybir.dt.float32

# vecs rows
V_NT1HI, V_NT1LO, V_A1, V_A2, V_S2, V_B2, V_RS2, V_NRS2 = range(8)
NVEC = 8

TRACE = False
LAST_RESULT = None

_cache: dict = {}


def _ternarize(nc, neg_hi, neg_lo, src, qtile, tmp_pool):
    """qtile = sign(src - hi) + sign(src - lo) in {-2..2}; /2 is folded into alpha.

    Two Scalar-engine Sign activations (per-partition bias APs) + one contiguous
    DVE add. Exact in bf16.
    """
    a = tmp_pool.tile([128, HW], QDT, tag="t1")
    b = tmp_pool.tile([128, HW], QDT, tag="t2")
    nc.scalar.activation(a, src, AFT.Sign, bias=neg_hi, scale=1.0)
    nc.scalar.activation(b, src, AFT.Sign, bias=neg_lo, scale=1.0)
    nc.vector.tensor_tensor(qtile, a, b, AOP.add)


def _zero_borders(nc, qtile):
    nc.gpsimd.memset(qtile[:, 0 : PW + 1], 0.0)                       # lead zero + row 0
    nc.gpsimd.memset(qtile[:, 1 + PW * 33 : 1 + PW * 34], 0.0)        # row 33
    side = qtile[:, 1 + PW : 1 + PW + 32 * PW].rearrange(
        "p (r c) -> p r c", c=PW
    )[:, :, 32:34]
    nc.gpsimd.memset(side, 0.0)                                       # cols 32/33


def _pad_interior(qtile):
    return qtile[:, 1 + PW : 1 + PW + 32 * PW].rearrange(
        "p (r c) -> p r c", c=PW
    )[:, :, 0:32]


def _conv_matmuls(nc, psum_tile, wtiles, qp, co, half):
    """Accumulate the 18 shifted-tap matmuls for one (co chunk, row half)."""
    idx = 0
    for kh in range(3):
        for kw in range(3):
            off = PW * (16 * half + kh) + kw
            for ci in range(CCH):
                rhs = qp[ci][:, off : off + PW * 16].rearrange(
                    "p (r c) -> p r c", c=PW
                )[:, :, 0:32]
                nc.tensor.matmul(
                    psum_tile,
                    lhsT=wtiles[kh * 3 + kw, ci][:, co * 128 : (co + 1) * 128],
                    rhs=rhs,
                    start=(idx == 0),
                    stop=(idx == 17),
                )
                idx += 1


def _build():
    if "nc" in _cache:
        return _cache["nc"]

    nc = bacc.Bacc("TRN2", num_devices=NCORES)

    x_in = nc.dram_tensor("x", (BL, CCH, 128, HW), F32, kind="ExternalInput")
    w1t = nc.dram_tensor("w1t", (9, CCH, 128, C), QDT, kind="ExternalInput")
    w2t = nc.dram_tensor("w2t", (9, CCH, 128, C), QDT, kind="ExternalInput")
    vecs = nc.dram_tensor("vecs", (NVEC, CCH, 128, 1), F32, kind="ExternalInput")
    out_d = nc.dram_tensor("out", (BL, CCH, 128, HW), F32, kind="ExternalOutput")
    cc_in = nc.dram_tensor("cc_in", (128, 1), F32)
    cc_out = nc.dram_tensor("cc_out", (128, 1), F32, addr_space="Shared")
    dsc = nc.dram_tensor("dsc", (1, 1), F32)

    with tile.TileContext(nc) as tc:
        with (
            tc.tile_pool(name="consts", bufs=1) as consts,
            tc.tile_pool(name="persist", bufs=1) as persist,
            tc.tile_pool(name="qpool", bufs=2) as qpool,
            tc.tile_pool(name="tmp", bufs=3) as tmp,
            tc.tile_pool(name="epi", bufs=4) as epi,
            tc.tile_pool(name="psum", bufs=8, space="PSUM") as psum,
        ):
            # ---- constants ----
            vt = {}
            for i in range(NVEC):
                for ci in range(CCH):
                    v = consts.tile([128, 1], F32, tag=f"v{i}_{ci}")
                    nc.sync.dma_start(out=v, in_=vecs[i, ci])
                    vt[i, ci] = v

            w1s, w2s = {}, {}
            for tap in range(9):
                for ci in range(CCH):
                    a = consts.tile([128, C], QDT, tag=f"w1_{tap}_{ci}")
                    nc.sync.dma_start(out=a, in_=w1t[tap, ci])
                    w1s[tap, ci] = a
                    b = consts.tile([128, C], QDT, tag=f"w2_{tap}_{ci}")
                    nc.sync.dma_start(out=b, in_=w2t[tap, ci])
                    w2s[tap, ci] = b

            partials = consts.tile([128, BL * CCH * 2], F32, tag="partials")

            xt, ht = {}, {}

            # ---------- phase A: block 1 + |bn2(h)| partial sums ----------
            pcol = 0
            for n in range(BL):
                for ci in range(CCH):
                    xtile = persist.tile([128, HW], F32, tag=f"x{n}_{ci}")
                    nc.sync.dma_start(out=xtile, in_=x_in[n, ci])
                    xt[n, ci] = xtile

                qp = {}
                for ci in range(CCH):
                    q = tmp.tile([128, HW], QDT, tag=f"qf{ci}")
                    _ternarize(nc, vt[V_NT1HI, ci], vt[V_NT1LO, ci], xt[n, ci], q, tmp)
                    qpad = qpool.tile([128, PBUF_AL], QDT, tag=f"qp{ci}")
                    _zero_borders(nc, qpad)
                    nc.sync.dma_start(out=_pad_interior(qpad), in_=q.rearrange("p (r c) -> p r c", c=32))
                    qp[ci] = qpad

                for co in range(CCH):
                    htile = persist.tile([128, HW], F32, tag=f"h{n}_{co}")
                    ht[n, co] = htile
                    for half in range(2):
                        ps = psum.tile([128, 512], F32, tag="ps")
                        _conv_matmuls(nc, ps, w1s, qp, co, half)
                        sl = slice(half * 512, (half + 1) * 512)
                        # h = alpha1 * conv + x   (one DVE op)
                        nc.vector.scalar_tensor_tensor(
                            out=htile[:, sl],
                            in0=ps,
                            scalar=vt[V_A1, co],
                            in1=xt[n, co][:, sl],
                            op0=AOP.mult,
                            op1=AOP.add,
                        )
                        # |bn2(h)| with per-partition running sum for delta2
                        zabs = tmp.tile([128, 512], F32, tag="zabs")
                        nc.scalar.activation(
                            out=zabs,
                            in_=htile[:, sl],
                            func=AFT.Abs,
                            bias=vt[V_B2, co],
                            scale=vt[V_S2, co],
                            accum_out=partials[:, pcol : pcol + 1],
                        )
                        pcol += 1

            # ---------- delta2 via cross-core AllReduce ----------
            ptot = consts.tile([128, 1], F32, tag="ptot")
            nc.vector.tensor_reduce(ptot, partials, axis=mybir.AxisListType.X, op=AOP.add)
            nc.sync.dma_start(out=cc_in[:], in_=ptot)
            nc.gpsimd.collective_compute(
                "AllReduce",
                AOP.add,
                replica_groups=[list(range(NCORES))],
                ins=[cc_in[:]],
                outs=[cc_out[:]],
            )
            # bring the 128 AllReduced values into one partition row, reduce to a
            # scalar, scale into delta2, then DMA-broadcast to all partitions
            red = consts.tile([1, 128], F32, tag="red")
            nc.sync.dma_start(out=red, in_=cc_out[:])
            drow = consts.tile([1, 1], F32, tag="drow")
            nc.vector.tensor_reduce(drow, red, axis=mybir.AxisListType.X, op=AOP.add)
            nc.vector.tensor_scalar(drow, drow, float(FRAC / (B * C * HW)), None, AOP.mult)
            nc.sync.dma_start(out=dsc[:], in_=drow)
            d2 = consts.tile([128, 1], F32, tag="d2")
            nc.sync.dma_start(out=d2, in_=dsc[:].to_broadcast((128, 1)))

            # per-chunk thresholds: t2hi = (d2 - b2)/s2 ; t2lo = (-d2 - b2)/s2
            # negated thresholds, used directly as Sign() biases:
            #   -t2hi = (b2 - d2)/s2 ;  -t2lo = (b2 + d2)/s2
            nt2hi, nt2lo = {}, {}
            for ci in range(CCH):
                thi = consts.tile([128, 1], F32, tag=f"nt2hi{ci}")
                nc.vector.tensor_tensor(thi, vt[V_B2, ci], d2, AOP.subtract)
                nc.vector.tensor_tensor(thi, thi, vt[V_RS2, ci], AOP.mult)
                nt2hi[ci] = thi
                tlo = consts.tile([128, 1], F32, tag=f"nt2lo{ci}")
                nc.vector.tensor_tensor(tlo, vt[V_B2, ci], d2, AOP.add)
                nc.vector.tensor_tensor(tlo, tlo, vt[V_RS2, ci], AOP.mult)
                nt2lo[ci] = tlo

            # ---------- phase B: block 2 ----------
            for n in range(BL):
                qp = {}
                for ci in range(CCH):
                    q = tmp.tile([128, HW], QDT, tag=f"qf{ci}")
                    _ternarize(nc, nt2hi[ci], nt2lo[ci], ht[n, ci], q, tmp)
                    qpad = qpool.tile([128, PBUF_AL], QDT, tag=f"qpb{ci}")
                    _zero_borders(nc, qpad)
                    nc.sync.dma_start(out=_pad_interior(qpad), in_=q.rearrange("p (r c) -> p r c", c=32))
                    qp[ci] = qpad

                for co in range(CCH):
                    for half in range(2):
                        ps = psum.tile([128, 512], F32, tag="ps")
                        _conv_matmuls(nc, ps, w2s, qp, co, half)
                        sl = slice(half * 512, (half + 1) * 512)
                        ot = epi.tile([128, 512], F32, tag="ot")
                        nc.vector.scalar_tensor_tensor(
                            out=ot,
                            in0=ps,
                            scalar=vt[V_A2, co],
                            in1=ht[n, co][:, sl],
                            op0=AOP.mult,
                            op1=AOP.add,
                        )
                        nc.sync.dma_start(out=out_d[n, co][:, sl], in_=ot)

    nc.finalize()
    _cache["nc"] = nc
    return nc


def _host_prep(x, w1, w2, gamma1, beta1, mean1, var1, gamma2, beta2, mean2, var2):
    f64 = np.float64
    npq = mybir.dt.np(QDT)

    s1 = (gamma1.astype(f64) / np.sqrt(var1.astype(f64) + EPS))
    b1 = beta1.astype(f64) - mean1.astype(f64) * s1
    assert (s1 > 0).all(), "kernel assumes positive bn scale (gamma>0)"
    # delta1 on host (f64 accumulate)
    z1 = x.astype(f64) * s1[None, :, None, None] + b1[None, :, None, None]
    d1 = FRAC * np.abs(z1).mean()
    t1hi = ((d1 - b1) / s1).astype(np.float32)
    t1lo = ((-d1 - b1) / s1).astype(np.float32)

    s2 = (gamma2.astype(f64) / np.sqrt(var2.astype(f64) + EPS))
    b2 = beta2.astype(f64) - mean2.astype(f64) * s2
    assert (s2 > 0).all(), "kernel assumes positive bn scale (gamma>0)"

    a1 = np.abs(w1.astype(f64)).mean(axis=(1, 2, 3)).astype(np.float32)
    a2 = np.abs(w2.astype(f64)).mean(axis=(1, 2, 3)).astype(np.float32)

    def wsign_t(w):
        # (O, I, 3, 3) -> (9, CCH, 128, C) with [tap, ci_chunk, k, co] = sign(w[co, ci, kh, kw])
        s = np.sign(w).astype(npq)
        return np.ascontiguousarray(
            s.transpose(2, 3, 1, 0).reshape(9, CCH, 128, C)
        )

    w1t = wsign_t(w1)
    w2t = wsign_t(w2)

    vecs = np.zeros((NVEC, CCH, 128, 1), np.float32)
    vecs[V_NT1HI] = (-t1hi).reshape(CCH, 128, 1)
    vecs[V_NT1LO] = (-t1lo).reshape(CCH, 128, 1)
    # q values are sign(.)+sign(.) in {-2..2}; fold the /2 into alpha
    vecs[V_A1] = (0.5 * a1).reshape(CCH, 128, 1)
    vecs[V_A2] = (0.5 * a2).reshape(CCH, 128, 1)
    vecs[V_S2] = s2.astype(np.float32).reshape(CCH, 128, 1)
    vecs[V_B2] = b2.astype(np.float32).reshape(CCH, 128, 1)
    vecs[V_RS2] = (1.0 / s2).astype(np.float32).reshape(CCH, 128, 1)
    vecs[V_NRS2] = (-1.0 / s2).astype(np.float32).reshape(CCH, 128, 1)
    return w1t, w2t, vecs


def make_in_maps(**inputs):
    x = np.ascontiguousarray(inputs["x"], np.float32)
    w1t, w2t, vecs = _host_prep(
        x,
        np.asarray(inputs["w1"], np.float32),
        np.asarray(inputs["w2"], np.float32),
        *[np.asarray(inputs[k], np.float32) for k in (
            "gamma1", "beta1", "mean1", "var1",
            "gamma2", "beta2", "mean2", "var2",
        )],
    )
    in_maps = []
    for i in range(NCORES):
        xs = np.ascontiguousarray(
            x[i * BL : (i + 1) * BL].reshape(BL, CCH, 128, HW)
        )
        in_maps.append({"x": xs, "w1t": w1t, "w2t": w2t, "vecs": vecs})
    return in_maps


def kernel(**inputs) -> np.ndarray:
    global LAST_RESULT
    nc = _build()
    in_maps = make_in_maps(**inputs)
    res = run_bass_kernel_spmd(nc, in_maps, list(range(NCORES)), trace=TRACE)
    LAST_RESULT = res
    out = np.concatenate(
        [res.results[i]["out"].reshape(BL, C, H, W) for i in range(NCORES)], axis=0
    )
    return out.astype(np.float32, copy=False)


# revision 12
# speedup vs baseline: 1.9055x; 1.2156x over previous
"""Trainium2 Bass kernel for nn_BasicBlockBi (TBN basic block, 2x ternary-binary conv).

Strategy: data-parallel over batch (4 images per core on 8 cores).
  - BN + ternarize thresholds for block 1 are folded on host into per-channel
    compare thresholds (delta1 computed on host from the full input).
  - Ternary activations {-1,0,+1} and binary sign weights are exact in bf16,
    so convs run as 3x3-tap shifted matmuls accumulating exact integers in PSUM;
    the per-output-channel alpha scale + shortcut add are fused in one DVE op.
  - delta2 = 0.7*mean|bn2(h)| needs a global mean over the full batch: per-core
    partial sums are AllReduced across the 8 cores in-kernel.
"""

import os
import sys

for _p in ("/opt/trn_rl_repo", "/root/.axon_site/_ro/trn_rl_repo"):
    if os.path.isdir(_p) and _p not in sys.path:
        sys.path.append(_p)

import numpy as np

import concourse.bacc as bacc
import concourse.bass as bass
import concourse.tile as tile
from concourse import bass_isa, mybir
from concourse.bass_utils import run_bass_kernel_spmd

B, C, H, W = 32, 256, 32, 32
HW = H * W
NCORES = 8
BL = B // NCORES          # images per core
CCH = C // 128            # channel chunks of 128
PW = W + 2                # padded row width
PBUF = 1 + PW * PW        # lead zero + 34x34 padded plane
PBUF_AL = PBUF + 3        # tail pad so over-slices stay in bounds
EPS = 1e-5
FRAC = 0.7

QMODE = "fp8"             # "fp8" (DoubleRow, 2x PE throughput) or "bf16"
QDT = mybir.dt.float8e4 if QMODE == "fp8" else mybir.dt.bfloat16  # exact for {-2..2}
SDT = mybir.dt.bfloat16   # Sign() activation output dtype

AOP = mybir.AluOpType
AFT = mybir.ActivationFunctionType
F32 = mybir.dt.float32

# vecs rows
V_NT1HI, V_NT1LO, V_A1, V_A2, V_S2, V_B2, V_RS2, V_NRS2 = range(8)
NVEC = 8

TRACE = False
LAST_RESULT = None

_cache: dict = {}


def _ternarize(nc, neg_hi, neg_lo, src, qtile, tmp_pool):
    """qtile = sign(src - hi) + sign(src - lo) in {-2..2}; /2 is folded into alpha.

    Two Scalar-engine Sign activations (per-partition bias APs) + one contiguous
    DVE add. Exact in bf16.
    """
    a = tmp_pool.tile([128, HW], SDT, tag="t1")
    b = tmp_pool.tile([128, HW], SDT, tag="t2")
    nc.scalar.activation(a, src, AFT.Sign, bias=neg_hi, scale=1.0)
    nc.scalar.activation(b, src, AFT.Sign, bias=neg_lo, scale=1.0)
    nc.vector.tensor_tensor(qtile, a, b, AOP.add)


def _zero_all(nc, qtile):
    nc.vector.memset(qtile[:], 0.0)


def _pad_interior(qtile):
    return qtile[:, 1 + PW : 1 + PW + 32 * PW].rearrange(
        "p (r c) -> p r c", c=PW
    )[:, :, 0:32]


def _conv_matmuls(nc, psum_tile, wtiles, qp, co, half):
    """Accumulate the shifted-tap matmuls for one (co chunk, row half)."""
    idx = 0
    if QMODE == "fp8":
        for kh in range(3):
            for kw in range(3):
                off = PW * (16 * half + kh) + kw
                rhs = qp[:, :, off : off + PW * 16].rearrange(
                    "p t (r c) -> p t r c", c=PW
                )[:, :, :, 0:32]
                nc.tensor.matmul(
                    psum_tile,
                    lhsT=wtiles[kh * 3 + kw][:, :, co * 128 : (co + 1) * 128],
                    rhs=rhs,
                    start=(idx == 0),
                    stop=(idx == 8),
                    perf_mode=mybir.MatmulPerfMode.DoubleRow,
                )
                idx += 1
    else:
        for kh in range(3):
            for kw in range(3):
                off = PW * (16 * half + kh) + kw
                for ci in range(CCH):
                    rhs = qp[ci][:, off : off + PW * 16].rearrange(
                        "p (r c) -> p r c", c=PW
                    )[:, :, 0:32]
                    nc.tensor.matmul(
                        psum_tile,
                        lhsT=wtiles[kh * 3 + kw, ci][:, co * 128 : (co + 1) * 128],
                        rhs=rhs,
                        start=(idx == 0),
                        stop=(idx == 17),
                    )
                    idx += 1


def _build():
    if "nc" in _cache:
        return _cache["nc"]

    nc = bacc.Bacc("TRN2", num_devices=NCORES)

    x_in = nc.dram_tensor("x", (BL, CCH, 128, HW), F32, kind="ExternalInput")
    # fp8: [tap, k, ci, co] (DoubleRow pairs ci along dim2); bf16: [tap, ci, k, co]
    wshape = (9, 128, CCH, C) if QMODE == "fp8" else (9, CCH, 128, C)
    w1t = nc.dram_tensor("w1t", wshape, QDT, kind="ExternalInput")
    w2t = nc.dram_tensor("w2t", wshape, QDT, kind="ExternalInput")
    vecs = nc.dram_tensor("vecs", (NVEC, CCH, 128, 1), F32, kind="ExternalInput")
    out_d = nc.dram_tensor("out", (BL, CCH, 128, HW), F32, kind="ExternalOutput")
    cc_in = nc.dram_tensor("cc_in", (128, 1), F32)
    cc_out = nc.dram_tensor("cc_out", (128, 1), F32, addr_space="Shared")
    dsc = nc.dram_tensor("dsc", (1, 1), F32)

    with tile.TileContext(nc) as tc:
        with (
            tc.tile_pool(name="consts", bufs=1) as consts,
            tc.tile_pool(name="persist", bufs=1) as persist,
            tc.tile_pool(name="tmp", bufs=3) as tmp,
            tc.tile_pool(name="epi", bufs=4) as epi,
            tc.tile_pool(name="psum", bufs=8, space="PSUM") as psum,
        ):
            # ---- constants ----
            vt = {}
            for i in range(NVEC):
                for ci in range(CCH):
                    v = consts.tile([128, 1], F32, tag=f"v{i}_{ci}")
                    nc.sync.dma_start(out=v, in_=vecs[i, ci])
                    vt[i, ci] = v

            w1s, w2s = {}, {}
            if QMODE == "fp8":
                for tap in range(9):
                    a = consts.tile([128, CCH, C], QDT, tag=f"w1_{tap}")
                    nc.sync.dma_start(out=a, in_=w1t[tap])
                    w1s[tap] = a
                    b = consts.tile([128, CCH, C], QDT, tag=f"w2_{tap}")
                    nc.sync.dma_start(out=b, in_=w2t[tap])
                    w2s[tap] = b
            else:
                for tap in range(9):
                    for ci in range(CCH):
                        a = consts.tile([128, C], QDT, tag=f"w1_{tap}_{ci}")
                        nc.sync.dma_start(out=a, in_=w1t[tap, ci])
                        w1s[tap, ci] = a
                        b = consts.tile([128, C], QDT, tag=f"w2_{tap}_{ci}")
                        nc.sync.dma_start(out=b, in_=w2t[tap, ci])
                        w2s[tap, ci] = b

            partials = consts.tile([128, BL * CCH * 2], F32, tag="partials")

            def make_qpads(prefix):
                pads = {}
                for par in range(2):
                    for ci_or_all in ([None] if QMODE == "fp8" else range(CCH)):
                        if QMODE == "fp8":
                            qq = consts.tile([128, CCH, PBUF_AL], QDT, tag=f"{prefix}{par}")
                        else:
                            qq = consts.tile([128, PBUF_AL], QDT, tag=f"{prefix}{par}_{ci_or_all}")
                        _zero_all(nc, qq)
                        pads[par, ci_or_all] = qq
                return pads

            qpadsA = make_qpads("qpA")
            qpadsB = make_qpads("qpB")

            xt, ht = {}, {}

            # ---------- phase A: block 1 + |bn2(h)| partial sums ----------
            pcol = 0
            for n in range(BL):
                for ci in range(CCH):
                    xtile = persist.tile([128, HW], F32, tag=f"x{n}_{ci}")
                    nc.sync.dma_start(out=xtile, in_=x_in[n, ci])
                    xt[n, ci] = xtile

                if QMODE == "fp8":
                    qp = qpadsA[n % 2, None]
                else:
                    qp = {ci: qpadsA[n % 2, ci] for ci in range(CCH)}
                for ci in range(CCH):
                    q = tmp.tile([128, HW], QDT, tag=f"qf{ci}")
                    _ternarize(nc, vt[V_NT1HI, ci], vt[V_NT1LO, ci], xt[n, ci], q, tmp)
                    dst = _pad_interior(qp[:, ci] if QMODE == "fp8" else qp[ci])
                    nc.sync.dma_start(out=dst, in_=q.rearrange("p (r c) -> p r c", c=32))

                for co in range(CCH):
                    htile = persist.tile([128, HW], F32, tag=f"h{n}_{co}")
                    ht[n, co] = htile
                    for half in range(2):
                        ps = psum.tile([128, 512], F32, tag="ps")
                        _conv_matmuls(nc, ps, w1s, qp, co, half)
                        sl = slice(half * 512, (half + 1) * 512)
                        # h = alpha1 * conv + x   (one DVE op)
                        nc.vector.scalar_tensor_tensor(
                            out=htile[:, sl],
                            in0=ps,
                            scalar=vt[V_A1, co],
                            in1=xt[n, co][:, sl],
                            op0=AOP.mult,
                            op1=AOP.add,
                        )
                        # |bn2(h)| with per-partition running sum for delta2
                        zabs = tmp.tile([128, 512], F32, tag="zabs")
                        nc.scalar.activation(
                            out=zabs,
                            in_=htile[:, sl],
                            func=AFT.Abs,
                            bias=vt[V_B2, co],
                            scale=vt[V_S2, co],
                            accum_out=partials[:, pcol : pcol + 1],
                        )
                        pcol += 1

            # ---------- delta2 via cross-core AllReduce ----------
            ptot = consts.tile([128, 1], F32, tag="ptot")
            nc.vector.tensor_reduce(ptot, partials, axis=mybir.AxisListType.X, op=AOP.add)
            nc.sync.dma_start(out=cc_in[:], in_=ptot)
            nc.gpsimd.collective_compute(
                "AllReduce",
                AOP.add,
                replica_groups=[list(range(NCORES))],
                ins=[cc_in[:]],
                outs=[cc_out[:]],
            )
            # bring the 128 AllReduced values into one partition row, reduce to a
            # scalar, scale into delta2, then DMA-broadcast to all partitions
            red = consts.tile([1, 128], F32, tag="red")
            nc.sync.dma_start(out=red, in_=cc_out[:])
            drow = consts.tile([1, 1], F32, tag="drow")
            nc.vector.tensor_reduce(drow, red, axis=mybir.AxisListType.X, op=AOP.add)
            nc.vector.tensor_scalar(drow, drow, float(FRAC / (B * C * HW)), None, AOP.mult)
            nc.sync.dma_start(out=dsc[:], in_=drow)
            d2 = consts.tile([128, 1], F32, tag="d2")
            nc.sync.dma_start(out=d2, in_=dsc[:].to_broadcast((128, 1)))

            # per-chunk thresholds: t2hi = (d2 - b2)/s2 ; t2lo = (-d2 - b2)/s2
            # negated thresholds, used directly as Sign() biases:
            #   -t2hi = (b2 - d2)/s2 ;  -t2lo = (b2 + d2)/s2
            nt2hi, nt2lo = {}, {}
            for ci in range(CCH):
                thi = consts.tile([128, 1], F32, tag=f"nt2hi{ci}")
                nc.vector.tensor_tensor(thi, vt[V_B2, ci], d2, AOP.subtract)
                nc.vector.tensor_tensor(thi, thi, vt[V_RS2, ci], AOP.mult)
                nt2hi[ci] = thi
                tlo = consts.tile([128, 1], F32, tag=f"nt2lo{ci}")
                nc.vector.tensor_tensor(tlo, vt[V_B2, ci], d2, AOP.add)
                nc.vector.tensor_tensor(tlo, tlo, vt[V_RS2, ci], AOP.mult)
                nt2lo[ci] = tlo

            # ---------- phase B: block 2 ----------
            for n in range(BL):
                if QMODE == "fp8":
                    qp = qpadsB[n % 2, None]
                else:
                    qp = {ci: qpadsB[n % 2, ci] for ci in range(CCH)}
                for ci in range(CCH):
                    q = tmp.tile([128, HW], QDT, tag=f"qf{ci}")
                    _ternarize(nc, nt2hi[ci], nt2lo[ci], ht[n, ci], q, tmp)
                    dst = _pad_interior(qp[:, ci] if QMODE == "fp8" else qp[ci])
                    nc.sync.dma_start(out=dst, in_=q.rearrange("p (r c) -> p r c", c=32))

                for co in range(CCH):
                    for half in range(2):
                        ps = psum.tile([128, 512], F32, tag="ps")
                        _conv_matmuls(nc, ps, w2s, qp, co, half)
                        sl = slice(half * 512, (half + 1) * 512)
                        ot = epi.tile([128, 512], F32, tag="ot")
                        nc.vector.scalar_tensor_tensor(
                            out=ot,
                            in0=ps,
                            scalar=vt[V_A2, co],
                            in1=ht[n, co][:, sl],
                            op0=AOP.mult,
                            op1=AOP.add,
                        )
                        nc.sync.dma_start(out=out_d[n, co][:, sl], in_=ot)

    nc.finalize()
    _cache["nc"] = nc
    return nc


def _host_prep(x, w1, w2, gamma1, beta1, mean1, var1, gamma2, beta2, mean2, var2):
    f64 = np.float64
    npq = mybir.dt.np(QDT)

    s1 = (gamma1.astype(f64) / np.sqrt(var1.astype(f64) + EPS))
    b1 = beta1.astype(f64) - mean1.astype(f64) * s1
    assert (s1 > 0).all(), "kernel assumes positive bn scale (gamma>0)"
    # delta1 on host (f64 accumulate)
    z1 = x.astype(f64) * s1[None, :, None, None] + b1[None, :, None, None]
    d1 = FRAC * np.abs(z1).mean()
    t1hi = ((d1 - b1) / s1).astype(np.float32)
    t1lo = ((-d1 - b1) / s1).astype(np.float32)

    s2 = (gamma2.astype(f64) / np.sqrt(var2.astype(f64) + EPS))
    b2 = beta2.astype(f64) - mean2.astype(f64) * s2
    assert (s2 > 0).all(), "kernel assumes positive bn scale (gamma>0)"

    a1 = np.abs(w1.astype(f64)).mean(axis=(1, 2, 3)).astype(np.float32)
    a2 = np.abs(w2.astype(f64)).mean(axis=(1, 2, 3)).astype(np.float32)

    def wsign_t(w):
        s = np.sign(w).astype(npq)
        # (O, I, 3, 3) -> (kh, kw, I, O)
        t = s.transpose(2, 3, 1, 0).reshape(9, CCH, 128, C)  # [tap, ci, k, co]
        if QMODE == "fp8":
            t = t.transpose(0, 2, 1, 3)  # [tap, k, ci, co]
        return np.ascontiguousarray(t)

    w1t = wsign_t(w1)
    w2t = wsign_t(w2)

    vecs = np.zeros((NVEC, CCH, 128, 1), np.float32)
    vecs[V_NT1HI] = (-t1hi).reshape(CCH, 128, 1)
    vecs[V_NT1LO] = (-t1lo).reshape(CCH, 128, 1)
    # q values are sign(.)+sign(.) in {-2..2}; fold the /2 into alpha
    vecs[V_A1] = (0.5 * a1).reshape(CCH, 128, 1)
    vecs[V_A2] = (0.5 * a2).reshape(CCH, 128, 1)
    vecs[V_S2] = s2.astype(np.float32).reshape(CCH, 128, 1)
    vecs[V_B2] = b2.astype(np.float32).reshape(CCH, 128, 1)
    vecs[V_RS2] = (1.0 / s2).astype(np.float32).reshape(CCH, 128, 1)
    vecs[V_NRS2] = (-1.0 / s2).astype(np.float32).reshape(CCH, 128, 1)
    return w1t, w2t, vecs


def make_in_maps(**inputs):
    x = np.ascontiguousarray(inputs["x"], np.float32)
    w1t, w2t, vecs = _host_prep(
        x,
        np.asarray(inputs["w1"], np.float32),
        np.asarray(inputs["w2"], np.float32),
        *[np.asarray(inputs[k], np.float32) for k in (
            "gamma1", "beta1", "mean1", "var1",
            "gamma2", "beta2", "mean2", "var2",
        )],
    )
    in_maps = []
    for i in range(NCORES):
        xs = np.ascontiguousarray(
            x[i * BL : (i + 1) * BL].reshape(BL, CCH, 128, HW)
        )
        in_maps.append({"x": xs, "w1t": w1t, "w2t": w2t, "vecs": vecs})
    return in_maps


def kernel(**inputs) -> np.ndarray:
    global LAST_RESULT
    nc = _build()
    in_maps = make_in_maps(**inputs)
    res = run_bass_kernel_spmd(nc, in_maps, list(range(NCORES)), trace=TRACE)
    LAST_RESULT = res
    out = np.concatenate(
        [res.results[i]["out"].reshape(BL, C, H, W) for i in range(NCORES)], axis=0
    )
    return out.astype(np.float32, copy=False)
